# revision 42
# baseline (speedup 1.0000x reference)
"""GTN (graph transformer network) forward on 8 Trainium2 cores.

Math (identical to the reference, right-associated):
  A_t = dense adjacency per edge type; A_i[c] = softmax(w_i)[c] . A
  H1 = A1@A2, H = rownorm(H1); out rows = rownorm(H@A3) @ XW.
  rownorm commutes through the left matmul, so
      rownorm(rownorm(A1@A2) @ A3) == rownorm(A1@A2@A3)
  and the chain right-associates:
      Z = A1 @ (A2 @ (A3 @ XW))         (~26 GFLOP instead of ~550)
  Only rows in target_x are read, so the last stage uses A1[target_x,:].
  The row normalizer rowsum(A1@A2@A3)[target_x] is the same chain applied
  to the ones vector; it is computed on the host as two sgemv's.

Sharding: channel c = core//4, row-quarter q = core%4; two AllGathers
(replica groups [0-3] and [4-7]) re-assemble the [4096,128] intermediates.

Device (per core); A matrices and gathered T3/T2 fp8-e4m3, XW bf16 (mixed
operand dtypes are supported by the PE; measured end-to-end rel err 5.5e-3
vs the 2e-2 gate), PSUM accumulation f32:
  stage A: T3^T[:, q] = (XW chunk_k)^T @ A3^T[chunk_k, cols_q]  (stationary=XW)
  transpose T3^T -> T3 via PE, AllGather -> T3 [4096,128]
  stage B: T2^T like stage A with stationary = T3 chunks, moving = A2^T strips
  transpose + AllGather -> T2 [4096,128]
  stage C: Z^T[:, targets_q] with stationary = T2 chunks, moving = A1^T strips
Big matrices stream from HBM as 32 row strips each so matmuls start after
the first strip, and the moving operand is 512 wide (LDWEIGHTS amortized).
The host ships everything pre-transposed (the adjacency is scattered
transposed at build time, so no transpose cost anywhere on the host).
"""

import os
import time
import numpy as np
from contextlib import ExitStack

NUM_EDGE = 5
C = 2
N = 4096
W_IN = 512
W_OUT = 128
NCORES = 8
P = 128
NK = N // P              # 32 contraction chunks
RQ = N // 4              # 1024 rows per core in stages A/B
NT = 1024                # n_target
TQ = NT // 4             # 256 target rows per core in stage C
DOUT = W_OUT             # 128
GROUPS = [[0, 1, 2, 3], [4, 5, 6, 7]]

_NC_CACHE = {}
LAST_EXEC_NS = None
LAST_RES = None
_LAST_IN_MAPS = None


def _build_nc():
    import concourse.tile as tile
    from concourse import bacc, mybir
    from concourse.masks import make_identity

    nc = bacc.Bacc("TRN2", target_bir_lowering=False, debug=False,
                   num_devices=NCORES)
    f32 = mybir.dt.float32
    bf16 = mybir.dt.bfloat16
    f8 = mybir.dt.float8e4

    a3t = nc.dram_tensor("a3t", [N, RQ], f8, kind="ExternalInput").ap()
    a2t = nc.dram_tensor("a2t", [N, RQ], f8, kind="ExternalInput").ap()
    a1t = nc.dram_tensor("a1t", [N, TQ], f8, kind="ExternalInput").ap()
    xw = nc.dram_tensor("xw", [N, DOUT], bf16, kind="ExternalInput").ap()
    z = nc.dram_tensor("z", [DOUT, TQ], f32, kind="ExternalOutput").ap()

    HQ = RQ // 2           # 512: half of a quarter
    with tile.TileContext(nc) as tc, ExitStack() as ctx:
        sbp = ctx.enter_context(tc.tile_pool(name="sbp", bufs=1))
        castp = ctx.enter_context(tc.tile_pool(name="castp", bufs=2))
        tpp = ctx.enter_context(tc.tile_pool(name="tpp", bufs=2))
        dram = ctx.enter_context(tc.tile_pool(name="dram", bufs=1, space="DRAM"))
        psacc = ctx.enter_context(tc.tile_pool(name="psacc", bufs=2, space="PSUM"))
        psc = ctx.enter_context(tc.tile_pool(name="psc", bufs=1, space="PSUM"))
        pstp = ctx.enter_context(tc.tile_pool(name="pstp", bufs=4, space="PSUM"))

        # big-matrix strips land in slices of one resident tile per matrix;
        # per-slice writes let each matmul depend only on its own strip DMA
        a3_sb = sbp.tile([P, NK * RQ], f8)
        a2_sb = sbp.tile([P, NK * RQ], f8)
        a1_sb = sbp.tile([P, NK * TQ], f8)
        xw_sb = sbp.tile([P, NK * DOUT], bf16)
        # gathered intermediates stored/gathered/reloaded as fp8: halves the
        # AllGather data phase and the reload on the critical path (end-to-end
        # rel err 5.2e-3 vs 3.8e-3 with bf16 payloads; gate is 2e-2)
        t3_sb = sbp.tile([P, NK * DOUT], f8)
        t2_sb = sbp.tile([P, NK * DOUT], f8)

        t3p_d = dram.tile([RQ, DOUT], f8)
        t3f_d = dram.tile([N, DOUT], f8)
        t2p_d = dram.tile([RQ, DOUT], f8)
        t2f_d = dram.tile([N, DOUT], f8)

        # xw first on the same queue as the a3 strips: stage A's first matmul
        # needs xw chunk 0 + strip 0, and sync's preamble finishes earliest
        nc.sync.dma_start(
            xw_sb[:].rearrange("p (k d) -> p k d", k=NK),
            xw.rearrange("(k p) d -> p k d", p=P))
        ident = sbp.tile([P, P], bf16)
        make_identity(nc, ident[:])
        # all strip loads serial on sync, in consumption order: one queue at
        # full HBM bandwidth beats two queues splitting it (stage A is the
        # head of the dependency chain and is piped behind the a3 strips).
        # Batches of 4 chunks per DMA keep pipelining while quartering the
        # semaphore traffic (and the teardown's per-sem epilogue cost).
        KB = 4                     # k-chunks per DMA batch
        for b in range(NK // KB):
            nc.sync.dma_start(
                a3_sb[:, b * KB * RQ:(b + 1) * KB * RQ]
                .rearrange("p (kk m) -> p kk m", kk=KB),
                a3t[b * KB * P:(b + 1) * KB * P, :]
                .rearrange("(kk p) m -> p kk m", p=P))
        for b in range(NK // KB):
            nc.sync.dma_start(
                a2_sb[:, b * KB * RQ:(b + 1) * KB * RQ]
                .rearrange("p (kk m) -> p kk m", kk=KB),
                a2t[b * KB * P:(b + 1) * KB * P, :]
                .rearrange("(kk p) m -> p kk m", p=P))
        for b in range(2):
            hk = NK // 2
            nc.sync.dma_start(
                a1_sb[:, b * hk * TQ:(b + 1) * hk * TQ]
                .rearrange("p (kk m) -> p kk m", kk=hk),
                a1t[b * hk * P:(b + 1) * hk * P, :]
                .rearrange("(kk p) m -> p kk m", p=P))

        def gather(pd, fd, dst_sb):
            nc.gpsimd.collective_compute(
                "AllGather", mybir.AluOpType.bypass,
                replica_groups=GROUPS, ins=[pd.opt()], outs=[fd.opt()])
            # reload split in 4 over two queues: it sits on the critical path
            # between the collective and the next stage's first matmul, which
            # only needs chunk 0 (first scalar piece)
            qk = NK // 4
            for j, eng in enumerate([nc.scalar, nc.scalar, nc.sync, nc.sync]):
                eng.dma_start(
                    dst_sb[:, j * qk * DOUT:(j + 1) * qk * DOUT]
                    .rearrange("p (i d) -> p i d", i=qk),
                    fd[j * qk * P:(j + 1) * qk * P, :]
                    .rearrange("(i p) d -> p i d", p=P))

        def stage_ab(stat_sb, mov_sb, name, pd, double_row=False):
            # interleaved halves ride the strip-DMA pipe and finish together;
            # cast/transpose/store tail follows, then the collective fires.
            # double_row (both operands fp8) packs two contraction chunks per
            # pass: out = stat[2k]^T@mov[2k] + stat[2k+1]^T@mov[2k+1] in one
            # matmul, halving the PE time of the stage
            accs = [psacc.tile([P, HQ], f32, tag="acc", name=f"acc{name}{h}")
                    for h in range(2)]
            stat3 = stat_sb[:].rearrange("p (k d) -> p k d", k=NK)
            mov3 = mov_sb[:].rearrange("p (k m) -> p k m", k=NK)
            if double_row:
                for k in range(NK // 2):
                    for h in range(2):
                        nc.tensor.matmul(
                            accs[h][:], stat3[:, 2 * k:2 * k + 2, :],
                            mov3[:, 2 * k:2 * k + 2, h * HQ:(h + 1) * HQ],
                            start=(k == 0), stop=(k == NK // 2 - 1),
                            perf_mode=mybir.MatmulPerfMode.DoubleRow,
                            skip_group_check=True)
            else:
                for k in range(NK):
                    for h in range(2):
                        nc.tensor.matmul(
                            accs[h][:], stat3[:, k, :],
                            mov3[:, k, h * HQ:(h + 1) * HQ],
                            start=(k == 0), stop=(k == NK - 1),
                            skip_group_check=True)
            stg = tpp.tile([P, RQ], f8, tag="tpstage")
            for h in range(2):
                cast = castp.tile([P, HQ], bf16, tag="cast", name=f"t{name}{h}")
                nc.vector.tensor_copy(cast[:], accs[h][:])
                for j in range(4):
                    pt = pstp.tile([P, P], bf16, tag="tp")
                    nc.tensor.transpose(pt[:], cast[:, j * P:(j + 1) * P],
                                        ident[:])
                    nc.vector.tensor_copy(
                        stg[:, (4 * h + j) * P:(4 * h + j + 1) * P], pt[:])
            nc.scalar.dma_start(
                pd[:].rearrange("(j p) d -> p j d", p=P),
                stg[:].rearrange("p (j d) -> p j d", j=8))

        # ---- stage A: T3^T = XW^T @ A3^T ----
        stage_ab(xw_sb, a3_sb, "A", t3p_d)
        gather(t3p_d, t3f_d, t3_sb)

        # ---- stage B: T2^T = T3^T @ A2^T (fp8 x fp8 -> DoubleRow) ----
        stage_ab(t3_sb, a2_sb, "B", t2p_d, double_row=True)
        gather(t2p_d, t2f_d, t2_sb)

        # ---- stage C: Z^T = T2^T @ A1^T (fp8 x fp8 -> DoubleRow) ----
        accC = psc.tile([P, TQ], f32, tag="accC")
        t2_3 = t2_sb[:].rearrange("p (k d) -> p k d", k=NK)
        a1_3 = a1_sb[:].rearrange("p (k m) -> p k m", k=NK)
        for k in range(NK // 2):
            nc.tensor.matmul(
                accC[:], t2_3[:, 2 * k:2 * k + 2, :],
                a1_3[:, 2 * k:2 * k + 2, :],
                start=(k == 0), stop=(k == NK // 2 - 1),
                perf_mode=mybir.MatmulPerfMode.DoubleRow,
                skip_group_check=True)
        zT = castp.tile([P, TQ], f32, tag="zout")
        nc.vector.tensor_copy(zT[:], accC[:])
        nc.sync.dma_start(z, zT[:])

    nc.compile()
    return nc


def _get_nc():
    if "nc" not in _NC_CACHE:
        _NC_CACHE["nc"] = _build_nc()
    return _NC_CACHE["nc"]


def _softmax_rows(w):
    w = np.asarray(w, np.float32)
    e = np.exp(w - w.max(axis=1, keepdims=True))
    return (e / e.sum(axis=1, keepdims=True)).astype(np.float32)


def _install_ntff_hook():
    """Recreate antenv.axon_hooks if the image lacks it (profiling only)."""
    import sys
    import types
    try:
        from antenv.axon_hooks import get_axon_ntff_profile_hook  # noqa: F401
        return
    except ImportError:
        pass
    try:
        from trn_agent_boot.trn_boot import _ntff_profile_via_ctypes
        import antenv
        mod = types.ModuleType("antenv.axon_hooks")
        state = {"h": None}
        mod.set_axon_ntff_profile_hook = lambda h: state.__setitem__("h", h)
        mod.get_axon_ntff_profile_hook = lambda: state["h"]
        sys.modules["antenv.axon_hooks"] = mod
        antenv.axon_hooks = mod
        mod.set_axon_ntff_profile_hook(
            _ntff_profile_via_ctypes("/opt/axon/libaxon_pjrt.so"))
    except Exception:
        pass


def _install_neff_cache():
    """Disk-cache the BIR->NEFF compile (keyed by exact BIR bytes)."""
    try:
        import hashlib
        import shutil
        import concourse.bass2jax as b2j
        if getattr(b2j, "_gtn_neff_cache", None):
            return
        orig = b2j.compile_bir_kernel
        root = "/var/tmp/gtn_neff_cache"

        def cached(bir_json, tmpdir, neff_name="file.neff"):
            d = None
            src = None
            try:
                key = hashlib.sha256(bir_json).hexdigest()[:32]
                d = os.path.join(root, key)
                src = os.path.join(d, "cached.neff")
                if os.path.exists(src):
                    dst = os.path.join(tmpdir, neff_name)
                    shutil.copy(src, dst)
                    return dst
            except Exception:
                pass
            out = orig(bir_json, tmpdir, neff_name)
            try:
                if src is not None:
                    os.makedirs(d, exist_ok=True)
                    tmp = src + f".tmp.{os.getpid()}"
                    shutil.copy(out, tmp)
                    os.replace(tmp, src)
            except Exception:
                pass
            return out

        b2j.compile_bir_kernel = cached
        b2j._gtn_neff_cache = True
    except Exception:
        pass


def kernel(edge_index, edge_value, X, target_x, w_l0_c1, w_l0_c2, w_l1_c1,
           gcn_w, gcn_b, lin_w, lin_b):
    global LAST_EXEC_NS, LAST_RES
    import ml_dtypes
    from concourse.bass_utils import run_bass_kernel_spmd
    bf16 = ml_dtypes.bfloat16
    fp8 = ml_dtypes.float8_e4m3fn

    tlog = (lambda msg, _t=[time.time()]:
            (print(f"[gtn] {msg}: {time.time() - _t[0]:.2f}s", flush=True),
             _t.__setitem__(0, time.time()))) \
        if os.environ.get("GTN_TIMING") else (lambda msg: None)

    src = np.asarray(edge_index[:, 0], np.int64)
    dst = np.asarray(edge_index[:, 1], np.int64)
    val = np.asarray(edge_value, np.float32)
    tx = np.asarray(target_x, np.int64)

    # transposed adjacency stack AT[t] = A_t^T (scatter with swapped indices;
    # duplicate edges accumulate)
    AT = np.zeros((NUM_EDGE, N * N), np.float32)
    for t in range(NUM_EDGE):
        np.add.at(AT[t], dst[t] * N + src[t], val[t])
    tlog("adj scatter")

    f1 = _softmax_rows(w_l0_c1)
    f2 = _softmax_rows(w_l0_c2)
    f3 = _softmax_rows(w_l1_c1)

    # full transposed combos for layers whose every row participates
    A2T = (f2 @ AT).reshape(C, N, N)
    A3T = (f3 @ AT).reshape(C, N, N)
    # stage C only ever reads the target rows of A1 == target cols of A1T
    G = AT.reshape(NUM_EDGE, N, N)[:, :, tx].reshape(NUM_EDGE, -1)
    A1G = (f1 @ G).reshape(C, N, NT)
    # rowsum(A_t) = colsum(AT_t), for the host-side normalizer chain
    cs = AT.reshape(NUM_EDGE, N, N).sum(axis=1)          # [T, N]
    AT = None
    G = None
    tlog("combos")

    # normalizer: s[c] = rowsum(A1@A2@A3)[target_x] via the same chain on 1s
    s = np.empty((C, NT), np.float32)
    for c in range(C):
        v3 = f3[c] @ cs                                   # rowsum(A3[c])
        v2 = v3 @ A2T[c]                                  # A2[c] @ v3
        s[c] = v2 @ A1G[c]                                # A1[c][tx,:] @ v2
    tlog("normalizers")

    XW = np.asarray(X, np.float32) @ np.asarray(gcn_w, np.float32)
    xw1 = XW.astype(bf16)

    in_maps = []
    for ci in range(NCORES):
        c, q = divmod(ci, 4)
        in_maps.append({
            "a3t": A3T[c][:, q * RQ:(q + 1) * RQ].astype(fp8),
            "a2t": A2T[c][:, q * RQ:(q + 1) * RQ].astype(fp8),
            "a1t": A1G[c][:, q * TQ:(q + 1) * TQ].astype(fp8),
            "xw": xw1,
        })
    tlog("bf16 shards")

    global _LAST_IN_MAPS
    _LAST_IN_MAPS = in_maps
    _install_neff_cache()
    nc = _get_nc()
    tlog("build+bass-compile")
    trace = bool(int(os.environ.get("GTN_TRACE", "1")))
    if trace:
        _install_ntff_hook()

    t0 = time.time()
    try:
        res = run_bass_kernel_spmd(nc, in_maps, list(range(NCORES)),
                                   trace=trace)
    except Exception:
        if not trace:
            raise
        t0 = time.time()
        res = run_bass_kernel_spmd(nc, in_maps, list(range(NCORES)),
                                   trace=False)
    wall_ns = int((time.time() - t0) * 1e9)
    LAST_EXEC_NS = res.exec_time_ns if res.exec_time_ns else wall_ns
    LAST_RES = res
    tlog("device run")

    gcn_b = np.asarray(gcn_b, np.float32)
    outs = []
    for c in range(C):
        Zt = np.concatenate([res.results[4 * c + q]["z"] for q in range(4)],
                            axis=1).T                     # [NT, 128]
        with np.errstate(divide="ignore", invalid="ignore"):
            sinv = np.where(s[c] == 0, 0.0, 1.0 / s[c]).astype(np.float32)
        outs.append(np.maximum(Zt * sinv[:, None] + gcn_b, 0.0))
    X_ = np.stack(outs, axis=1).reshape(NT, C * W_OUT)
    y = X_ @ np.asarray(lin_w, np.float32) + np.asarray(lin_b, np.float32)
    return y.astype(np.float32)


# revision 43
# speedup vs baseline: 1.1767x; 1.1767x over previous
"""GTN (graph transformer network) forward on 8 Trainium2 cores.

Math (identical to the reference, right-associated):
  A_t = dense adjacency per edge type; A_i[c] = softmax(w_i)[c] . A
  H1 = A1@A2, H = rownorm(H1); out rows = rownorm(H@A3) @ XW.
  rownorm commutes through the left matmul, so
      rownorm(rownorm(A1@A2) @ A3) == rownorm(A1@A2@A3)
  and the chain right-associates:
      Z = A1 @ (A2 @ (A3 @ XW))         (~26 GFLOP instead of ~550)
  Only rows in target_x are read, so the last stage uses A1[target_x,:].
  The row normalizer rowsum(A1@A2@A3)[target_x] is the same chain applied
  to the ones vector; it is computed on the host as two sgemv's.

Sharding: channel c = core//4, row-quarter q = core%4; two AllGathers
(replica groups [0-3] and [4-7]) re-assemble the [4096,128] intermediates.

Device (per core); A matrices and gathered T3/T2 fp8-e4m3, XW bf16 (mixed
operand dtypes are supported by the PE; measured end-to-end rel err 5.5e-3
vs the 2e-2 gate), PSUM accumulation f32:
  stage A: T3^T[:, q] = (XW chunk_k)^T @ A3^T[chunk_k, cols_q]  (stationary=XW)
  transpose T3^T -> T3 via PE, AllGather -> T3 [4096,128]
  stage B: T2^T like stage A with stationary = T3 chunks, moving = A2^T strips
  transpose + AllGather -> T2 [4096,128]
  stage C: Z^T[:, targets_q] with stationary = T2 chunks, moving = A1^T strips
Stages B and C run in MatmulPerfMode.DoubleRow (both operands fp8): two
contraction chunks packed per pass, halving their PE time.
Big matrices stream from HBM as 32 row strips each so matmuls start after
the first strip, and the moving operand is 512 wide (LDWEIGHTS amortized).
The host ships everything pre-transposed (the adjacency is scattered
transposed at build time, so no transpose cost anywhere on the host).
"""

import os
import time
import numpy as np
from contextlib import ExitStack

NUM_EDGE = 5
C = 2
N = 4096
W_IN = 512
W_OUT = 128
NCORES = 8
P = 128
NK = N // P              # 32 contraction chunks
RQ = N // 4              # 1024 rows per core in stages A/B
NT = 1024                # n_target
TQ = NT // 4             # 256 target rows per core in stage C
DOUT = W_OUT             # 128
GROUPS = [[0, 1, 2, 3], [4, 5, 6, 7]]

_NC_CACHE = {}
LAST_EXEC_NS = None
LAST_RES = None
_LAST_IN_MAPS = None


def _build_nc():
    import concourse.tile as tile
    from concourse import bacc, mybir
    from concourse.masks import make_identity

    nc = bacc.Bacc("TRN2", target_bir_lowering=False, debug=False,
                   num_devices=NCORES)
    f32 = mybir.dt.float32
    bf16 = mybir.dt.bfloat16
    f8 = mybir.dt.float8e4

    a3t = nc.dram_tensor("a3t", [N, RQ], f8, kind="ExternalInput").ap()
    a2t = nc.dram_tensor("a2t", [N, RQ], f8, kind="ExternalInput").ap()
    a1t = nc.dram_tensor("a1t", [N, TQ], f8, kind="ExternalInput").ap()
    xw = nc.dram_tensor("xw", [N, DOUT], bf16, kind="ExternalInput").ap()
    z = nc.dram_tensor("z", [DOUT, TQ], f32, kind="ExternalOutput").ap()

    HQ = RQ // 2           # 512: half of a quarter
    with tile.TileContext(nc) as tc, ExitStack() as ctx:
        sbp = ctx.enter_context(tc.tile_pool(name="sbp", bufs=1))
        castp = ctx.enter_context(tc.tile_pool(name="castp", bufs=2))
        tpp = ctx.enter_context(tc.tile_pool(name="tpp", bufs=2))
        dram = ctx.enter_context(tc.tile_pool(name="dram", bufs=1, space="DRAM"))
        psacc = ctx.enter_context(tc.tile_pool(name="psacc", bufs=2, space="PSUM"))
        psc = ctx.enter_context(tc.tile_pool(name="psc", bufs=1, space="PSUM"))
        pstp = ctx.enter_context(tc.tile_pool(name="pstp", bufs=4, space="PSUM"))

        # big-matrix strips land in slices of one resident tile per matrix;
        # per-slice writes let each matmul depend only on its own strip DMA
        a3_sb = sbp.tile([P, NK * RQ], f8)
        a2_sb = sbp.tile([P, NK * RQ], f8)
        a1_sb = sbp.tile([P, NK * TQ], f8)
        xw_sb = sbp.tile([P, NK * DOUT], bf16)
        # gathered intermediates stored/gathered/reloaded as fp8: halves the
        # AllGather data phase and the reload on the critical path (end-to-end
        # rel err 5.2e-3 vs 3.8e-3 with bf16 payloads; gate is 2e-2)
        t3_sb = sbp.tile([P, NK * DOUT], f8)
        t2_sb = sbp.tile([P, NK * DOUT], f8)

        t3p_d = dram.tile([RQ, DOUT], f8)
        t3f_d = dram.tile([N, DOUT], f8)
        t2p_d = dram.tile([RQ, DOUT], f8)
        t2f_d = dram.tile([N, DOUT], f8)

        # xw first on the same queue as the a3 strips: stage A's first matmul
        # needs xw chunk 0 + strip 0, and sync's preamble finishes earliest
        nc.sync.dma_start(
            xw_sb[:].rearrange("p (k d) -> p k d", k=NK),
            xw.rearrange("(k p) d -> p k d", p=P))
        ident = sbp.tile([P, P], bf16)
        make_identity(nc, ident[:])
        # all strip loads serial on sync, in consumption order: one queue at
        # full HBM bandwidth beats two queues splitting it (stage A is the
        # head of the dependency chain and is piped behind the a3 strips).
        # Batches of 4 chunks per DMA keep pipelining while quartering the
        # semaphore traffic (and the teardown's per-sem epilogue cost).
        KB = 4                     # k-chunks per DMA batch
        for b in range(NK // KB):
            nc.sync.dma_start(
                a3_sb[:, b * KB * RQ:(b + 1) * KB * RQ]
                .rearrange("p (kk m) -> p kk m", kk=KB),
                a3t[b * KB * P:(b + 1) * KB * P, :]
                .rearrange("(kk p) m -> p kk m", p=P))
        for b in range(NK // KB):
            nc.sync.dma_start(
                a2_sb[:, b * KB * RQ:(b + 1) * KB * RQ]
                .rearrange("p (kk m) -> p kk m", kk=KB),
                a2t[b * KB * P:(b + 1) * KB * P, :]
                .rearrange("(kk p) m -> p kk m", p=P))
        for b in range(2):
            hk = NK // 2
            nc.sync.dma_start(
                a1_sb[:, b * hk * TQ:(b + 1) * hk * TQ]
                .rearrange("p (kk m) -> p kk m", kk=hk),
                a1t[b * hk * P:(b + 1) * hk * P, :]
                .rearrange("(kk p) m -> p kk m", p=P))

        def gather(pd, fd, dst_sb):
            nc.gpsimd.collective_compute(
                "AllGather", mybir.AluOpType.bypass,
                replica_groups=GROUPS, ins=[pd.opt()], outs=[fd.opt()])
            # reload split in 4 over two queues: it sits on the critical path
            # between the collective and the next stage's first matmul, which
            # only needs chunk 0 (first scalar piece)
            qk = NK // 4
            for j, eng in enumerate([nc.scalar, nc.scalar, nc.sync, nc.sync]):
                eng.dma_start(
                    dst_sb[:, j * qk * DOUT:(j + 1) * qk * DOUT]
                    .rearrange("p (i d) -> p i d", i=qk),
                    fd[j * qk * P:(j + 1) * qk * P, :]
                    .rearrange("(i p) d -> p i d", p=P))

        def stage_ab(stat_sb, mov_sb, name, pd, double_row=False):
            # interleaved halves ride the strip-DMA pipe and finish together;
            # cast/transpose/store tail follows, then the collective fires.
            # double_row (both operands fp8) packs two contraction chunks per
            # pass: out = stat[2k]^T@mov[2k] + stat[2k+1]^T@mov[2k+1] in one
            # matmul, halving the PE time of the stage
            accs = [psacc.tile([P, HQ], f32, tag="acc", name=f"acc{name}{h}")
                    for h in range(2)]
            stat3 = stat_sb[:].rearrange("p (k d) -> p k d", k=NK)
            mov3 = mov_sb[:].rearrange("p (k m) -> p k m", k=NK)
            if double_row:
                for k in range(NK // 2):
                    for h in range(2):
                        nc.tensor.matmul(
                            accs[h][:], stat3[:, 2 * k:2 * k + 2, :],
                            mov3[:, 2 * k:2 * k + 2, h * HQ:(h + 1) * HQ],
                            start=(k == 0), stop=(k == NK // 2 - 1),
                            perf_mode=mybir.MatmulPerfMode.DoubleRow,
                            skip_group_check=True)
            else:
                for k in range(NK):
                    for h in range(2):
                        nc.tensor.matmul(
                            accs[h][:], stat3[:, k, :],
                            mov3[:, k, h * HQ:(h + 1) * HQ],
                            start=(k == 0), stop=(k == NK - 1),
                            skip_group_check=True)
            stg = tpp.tile([P, RQ], f8, tag="tpstage")
            for h in range(2):
                cast = castp.tile([P, HQ], bf16, tag="cast", name=f"t{name}{h}")
                nc.vector.tensor_copy(cast[:], accs[h][:])
                for j in range(4):
                    pt = pstp.tile([P, P], bf16, tag="tp")
                    nc.tensor.transpose(pt[:], cast[:, j * P:(j + 1) * P],
                                        ident[:])
                    nc.vector.tensor_copy(
                        stg[:, (4 * h + j) * P:(4 * h + j + 1) * P], pt[:])
            nc.scalar.dma_start(
                pd[:].rearrange("(j p) d -> p j d", p=P),
                stg[:].rearrange("p (j d) -> p j d", j=8))

        # ---- stage A: T3^T = XW^T @ A3^T ----
        stage_ab(xw_sb, a3_sb, "A", t3p_d)
        gather(t3p_d, t3f_d, t3_sb)

        # ---- stage B: T2^T = T3^T @ A2^T (fp8 x fp8 -> DoubleRow) ----
        stage_ab(t3_sb, a2_sb, "B", t2p_d, double_row=True)
        gather(t2p_d, t2f_d, t2_sb)

        # ---- stage C: Z^T = T2^T @ A1^T (fp8 x fp8 -> DoubleRow) ----
        accC = psc.tile([P, TQ], f32, tag="accC")
        t2_3 = t2_sb[:].rearrange("p (k d) -> p k d", k=NK)
        a1_3 = a1_sb[:].rearrange("p (k m) -> p k m", k=NK)
        for k in range(NK // 2):
            nc.tensor.matmul(
                accC[:], t2_3[:, 2 * k:2 * k + 2, :],
                a1_3[:, 2 * k:2 * k + 2, :],
                start=(k == 0), stop=(k == NK // 2 - 1),
                perf_mode=mybir.MatmulPerfMode.DoubleRow,
                skip_group_check=True)
        zT = castp.tile([P, TQ], f32, tag="zout")
        nc.vector.tensor_copy(zT[:], accC[:])
        nc.sync.dma_start(z, zT[:])

    nc.compile()
    return nc


def _get_nc():
    if "nc" not in _NC_CACHE:
        _NC_CACHE["nc"] = _build_nc()
    return _NC_CACHE["nc"]


def _softmax_rows(w):
    w = np.asarray(w, np.float32)
    e = np.exp(w - w.max(axis=1, keepdims=True))
    return (e / e.sum(axis=1, keepdims=True)).astype(np.float32)


def _install_ntff_hook():
    """Recreate antenv.axon_hooks if the image lacks it (profiling only)."""
    import sys
    import types
    try:
        from antenv.axon_hooks import get_axon_ntff_profile_hook  # noqa: F401
        return
    except ImportError:
        pass
    try:
        from trn_agent_boot.trn_boot import _ntff_profile_via_ctypes
        import antenv
        mod = types.ModuleType("antenv.axon_hooks")
        state = {"h": None}
        mod.set_axon_ntff_profile_hook = lambda h: state.__setitem__("h", h)
        mod.get_axon_ntff_profile_hook = lambda: state["h"]
        sys.modules["antenv.axon_hooks"] = mod
        antenv.axon_hooks = mod
        mod.set_axon_ntff_profile_hook(
            _ntff_profile_via_ctypes("/opt/axon/libaxon_pjrt.so"))
    except Exception:
        pass


def _install_neff_cache():
    """Disk-cache the BIR->NEFF compile (keyed by exact BIR bytes)."""
    try:
        import hashlib
        import shutil
        import concourse.bass2jax as b2j
        if getattr(b2j, "_gtn_neff_cache", None):
            return
        orig = b2j.compile_bir_kernel
        root = "/var/tmp/gtn_neff_cache"

        def cached(bir_json, tmpdir, neff_name="file.neff"):
            d = None
            src = None
            try:
                key = hashlib.sha256(bir_json).hexdigest()[:32]
                d = os.path.join(root, key)
                src = os.path.join(d, "cached.neff")
                if os.path.exists(src):
                    dst = os.path.join(tmpdir, neff_name)
                    shutil.copy(src, dst)
                    return dst
            except Exception:
                pass
            out = orig(bir_json, tmpdir, neff_name)
            try:
                if src is not None:
                    os.makedirs(d, exist_ok=True)
                    tmp = src + f".tmp.{os.getpid()}"
                    shutil.copy(out, tmp)
                    os.replace(tmp, src)
            except Exception:
                pass
            return out

        b2j.compile_bir_kernel = cached
        b2j._gtn_neff_cache = True
    except Exception:
        pass


def kernel(edge_index, edge_value, X, target_x, w_l0_c1, w_l0_c2, w_l1_c1,
           gcn_w, gcn_b, lin_w, lin_b):
    global LAST_EXEC_NS, LAST_RES
    import ml_dtypes
    from concourse.bass_utils import run_bass_kernel_spmd
    bf16 = ml_dtypes.bfloat16
    fp8 = ml_dtypes.float8_e4m3fn

    tlog = (lambda msg, _t=[time.time()]:
            (print(f"[gtn] {msg}: {time.time() - _t[0]:.2f}s", flush=True),
             _t.__setitem__(0, time.time()))) \
        if os.environ.get("GTN_TIMING") else (lambda msg: None)

    src = np.asarray(edge_index[:, 0], np.int64)
    dst = np.asarray(edge_index[:, 1], np.int64)
    val = np.asarray(edge_value, np.float32)
    tx = np.asarray(target_x, np.int64)

    # transposed adjacency stack AT[t] = A_t^T (scatter with swapped indices;
    # duplicate edges accumulate)
    AT = np.zeros((NUM_EDGE, N * N), np.float32)
    for t in range(NUM_EDGE):
        np.add.at(AT[t], dst[t] * N + src[t], val[t])
    tlog("adj scatter")

    f1 = _softmax_rows(w_l0_c1)
    f2 = _softmax_rows(w_l0_c2)
    f3 = _softmax_rows(w_l1_c1)

    # full transposed combos for layers whose every row participates
    A2T = (f2 @ AT).reshape(C, N, N)
    A3T = (f3 @ AT).reshape(C, N, N)
    # stage C only ever reads the target rows of A1 == target cols of A1T
    G = AT.reshape(NUM_EDGE, N, N)[:, :, tx].reshape(NUM_EDGE, -1)
    A1G = (f1 @ G).reshape(C, N, NT)
    # rowsum(A_t) = colsum(AT_t), for the host-side normalizer chain
    cs = AT.reshape(NUM_EDGE, N, N).sum(axis=1)          # [T, N]
    AT = None
    G = None
    tlog("combos")

    # normalizer: s[c] = rowsum(A1@A2@A3)[target_x] via the same chain on 1s
    s = np.empty((C, NT), np.float32)
    for c in range(C):
        v3 = f3[c] @ cs                                   # rowsum(A3[c])
        v2 = v3 @ A2T[c]                                  # A2[c] @ v3
        s[c] = v2 @ A1G[c]                                # A1[c][tx,:] @ v2
    tlog("normalizers")

    XW = np.asarray(X, np.float32) @ np.asarray(gcn_w, np.float32)
    xw1 = XW.astype(bf16)

    in_maps = []
    for ci in range(NCORES):
        c, q = divmod(ci, 4)
        in_maps.append({
            "a3t": A3T[c][:, q * RQ:(q + 1) * RQ].astype(fp8),
            "a2t": A2T[c][:, q * RQ:(q + 1) * RQ].astype(fp8),
            "a1t": A1G[c][:, q * TQ:(q + 1) * TQ].astype(fp8),
            "xw": xw1,
        })
    tlog("bf16 shards")

    global _LAST_IN_MAPS
    _LAST_IN_MAPS = in_maps
    _install_neff_cache()
    nc = _get_nc()
    tlog("build+bass-compile")
    trace = bool(int(os.environ.get("GTN_TRACE", "1")))
    if trace:
        _install_ntff_hook()

    t0 = time.time()
    try:
        res = run_bass_kernel_spmd(nc, in_maps, list(range(NCORES)),
                                   trace=trace)
    except Exception:
        if not trace:
            raise
        t0 = time.time()
        res = run_bass_kernel_spmd(nc, in_maps, list(range(NCORES)),
                                   trace=False)
    wall_ns = int((time.time() - t0) * 1e9)
    LAST_EXEC_NS = res.exec_time_ns if res.exec_time_ns else wall_ns
    LAST_RES = res
    tlog("device run")

    gcn_b = np.asarray(gcn_b, np.float32)
    outs = []
    for c in range(C):
        Zt = np.concatenate([res.results[4 * c + q]["z"] for q in range(4)],
                            axis=1).T                     # [NT, 128]
        with np.errstate(divide="ignore", invalid="ignore"):
            sinv = np.where(s[c] == 0, 0.0, 1.0 / s[c]).astype(np.float32)
        outs.append(np.maximum(Zt * sinv[:, None] + gcn_b, 0.0))
    X_ = np.stack(outs, axis=1).reshape(NT, C * W_OUT)
    y = X_ @ np.asarray(lin_w, np.float32) + np.asarray(lin_b, np.float32)
    return y.astype(np.float32)
